# revision 2
# baseline (speedup 1.0000x reference)
"""Trainium2 Bass kernel for nn_CoreferenceModel (sparse_attention).

Strategy (8 NeuronCores, SPMD, no collectives):
  NEFF 1 (mention scorer): shard the N=2000 spans 8x250. Each core computes
    scores = relu(v @ Wm1 + bm1) @ wm2 + bm2 for its 250 spans (fp32 matmuls,
    D on the contraction/partition axis). Host gathers [2000] scores.
  Host: top-k (m=400) by score + lexsort by (start, end) descending —
    tiny integer work that is inherently sequential.
  NEFF 2 (pairwise scorer + softmax): shard the 399 output rows 8x50. The
    antecedent window of row i is the contiguous span block i+1..i+250 in
    sorted order, so each core gets a 300-column slice of v^T and computes
      h   = relu((a@Wa) + (v@Wb)[win] + (a*b)@Wab + bp1)   (fp16 matmuls)
      ant = h @ wp2 (+ shift-invariant constants) ; softmax over [250 + dummy]
    with (v@Wb), (a@Wa)+bp1 precomputed per-core, windows as pure slices.
    Rows are processed in pairs so the big matmuls stream N=500 columns.

Softmax shift trick: softmax(x + c) == softmax(x), so the per-row constant
(s_i + bp2) is not added to the 250 antecedent logits; instead the dummy
column becomes -(s_i + bp2). s-window values arrive pre-gathered (+ -1e30
mask) as a host-staged tensor; the adds happen on-device.
"""

import time

import numpy as np

D = 768
H = 1024
K = 250
N_SPANS = 2000
NCORES = 8
M = 400            # int(0.4 * 1000) retained spans
R = 50             # output rows per core (last core's row 399 is discarded)
W = R + K          # v^T columns per core in NEFF 2
NS = N_SPANS // NCORES  # spans per core in NEFF 1
DC = D // 128      # contraction chunks
HC = H // 128      # hidden chunks
NEG = -1.0e30

LAST_INFO: dict = {}
_CACHE: dict = {}


def _mybir():
    from concourse import mybir

    return mybir


def build_scores_nc():
    import concourse.bacc as bacc
    import concourse.tile as tile

    mybir = _mybir()
    f32 = mybir.dt.float32

    nc = bacc.Bacc("TRN2", target_bir_lowering=False)
    vt = nc.dram_tensor("vt", [128, DC, NS], f32, kind="ExternalInput")
    wm1 = nc.dram_tensor("wm1", [128, DC, H], f32, kind="ExternalInput")
    bm1h = nc.dram_tensor("bm1h", [128, HC], f32, kind="ExternalInput")
    wm2h = nc.dram_tensor("wm2h", [128, HC], f32, kind="ExternalInput")
    bm2s = nc.dram_tensor("bm2s", [1, 1], f32, kind="ExternalInput")
    out = nc.dram_tensor("out", [1, NS], f32, kind="ExternalOutput")

    with tile.TileContext(nc) as tc:
        with (
            tc.tile_pool(name="singles", bufs=1) as singles,
            tc.tile_pool(name="work", bufs=2) as work,
            tc.tile_pool(name="psh", bufs=2, space="PSUM") as psh,
            tc.tile_pool(name="pss", bufs=1, space="PSUM") as pss,
        ):
            sb_v = singles.tile([128, DC, NS], f32)
            nc.sync.dma_start(out=sb_v, in_=vt.ap())
            sb_w = singles.tile([128, DC, H], f32)
            nc.sync.dma_start(out=sb_w, in_=wm1.ap())
            sb_bm1 = singles.tile([128, HC], f32)
            nc.sync.dma_start(out=sb_bm1, in_=bm1h.ap())
            sb_wm2 = singles.tile([128, HC], f32)
            nc.sync.dma_start(out=sb_wm2, in_=wm2h.ap())
            sb_bm2 = singles.tile([1, 1], f32)
            nc.sync.dma_start(out=sb_bm2, in_=bm2s.ap())

            h_sb = singles.tile([128, HC, NS], f32)
            for hc in range(HC):
                ps = psh.tile([128, NS], f32)
                for dc in range(DC):
                    nc.tensor.matmul(
                        ps,
                        sb_w[:, dc, 128 * hc : 128 * (hc + 1)],
                        sb_v[:, dc, :],
                        start=(dc == 0),
                        stop=(dc == DC - 1),
                    )
                nc.scalar.activation(
                    out=h_sb[:, hc, :],
                    in_=ps,
                    func=mybir.ActivationFunctionType.Relu,
                    bias=sb_bm1[:, hc : hc + 1],
                    scale=1.0,
                )
            ps2 = pss.tile([1, NS], f32)
            for hc in range(HC):
                nc.tensor.matmul(
                    ps2,
                    sb_wm2[:, hc : hc + 1],
                    h_sb[:, hc, :],
                    start=(hc == 0),
                    stop=(hc == HC - 1),
                )
            o_sb = work.tile([1, NS], f32)
            nc.vector.tensor_scalar_add(o_sb, ps2, sb_bm2[0:1, 0:1])
            nc.sync.dma_start(out=out.ap(), in_=o_sb)
    nc.compile()
    return nc


def build_pair_nc():
    import concourse.bacc as bacc
    import concourse.tile as tile

    mybir = _mybir()
    f32 = mybir.dt.float32
    f16 = mybir.dt.float16
    AT = mybir.AluOpType

    nc = bacc.Bacc("TRN2", target_bir_lowering=False)
    vt = nc.dram_tensor("vt", [128, DC, W], f32, kind="ExternalInput")
    wab = nc.dram_tensor("wab", [128, DC, H], f16, kind="ExternalInput")
    wa = nc.dram_tensor("wa", [128, DC, H], f16, kind="ExternalInput")
    wb = nc.dram_tensor("wb", [128, DC, H], f16, kind="ExternalInput")
    bp1h = nc.dram_tensor("bp1h", [128, HC], f32, kind="ExternalInput")
    wp2h = nc.dram_tensor("wp2h", [128, HC], f16, kind="ExternalInput")
    bp2s = nc.dram_tensor("bp2s", [1, 1], f32, kind="ExternalInput")
    srow = nc.dram_tensor("srow", [1, R], f32, kind="ExternalInput")
    addt = nc.dram_tensor("addt", [1, R, K], f32, kind="ExternalInput")
    out = nc.dram_tensor("out", [R, K + 1], f32, kind="ExternalOutput")

    with tile.TileContext(nc) as tc:
        with (
            tc.tile_pool(name="singles", bufs=1) as singles,
            tc.tile_pool(name="rpool", bufs=3) as rpool,
            tc.tile_pool(name="hpool", bufs=2) as hpool,
            tc.tile_pool(name="tpool", bufs=3) as tpool,
            tc.tile_pool(name="spool", bufs=3) as spool,
            tc.tile_pool(name="opool", bufs=4) as opool,
            tc.tile_pool(name="psh", bufs=2, space="PSUM") as psh,
            tc.tile_pool(name="psr", bufs=2, space="PSUM") as psr_pool,
        ):
            sb_vt = singles.tile([128, DC, W], f32)
            nc.sync.dma_start(out=sb_vt, in_=vt.ap())
            sb_wa = singles.tile([128, DC, H], f16)
            nc.sync.dma_start(out=sb_wa, in_=wa.ap())
            sb_wb = singles.tile([128, DC, H], f16)
            nc.sync.dma_start(out=sb_wb, in_=wb.ap())
            sb_wab = singles.tile([128, DC, H], f16)
            nc.sync.dma_start(out=sb_wab, in_=wab.ap())
            sb_bp1 = singles.tile([128, HC], f32)
            nc.sync.dma_start(out=sb_bp1, in_=bp1h.ap())
            sb_wp2 = singles.tile([128, HC], f16)
            nc.sync.dma_start(out=sb_wp2, in_=wp2h.ap())
            sb_bp2 = singles.tile([1, 1], f32)
            nc.sync.dma_start(out=sb_bp2, in_=bp2s.ap())
            sb_srow = singles.tile([1, R], f32)
            nc.sync.dma_start(out=sb_srow, in_=srow.ap())
            sb_addt = singles.tile([1, R, K], f32)
            nc.sync.dma_start(out=sb_addt, in_=addt.ap())

            # fp16 copy of v^T for the precompute matmuls
            sb_vt16 = singles.tile([128, DC, W], f16)
            for dc in range(DC):
                nc.vector.tensor_copy(out=sb_vt16[:, dc, :], in_=sb_vt[:, dc, :])

            # negd[li] = -(s_i + bp2): value of the dummy column after the
            # shift-invariant removal of (s_i + bp2) from every logit.
            sb_negd = singles.tile([1, R], f32)
            nc.vector.tensor_scalar(
                sb_negd, sb_srow, sb_bp2[0:1, 0:1], -1.0, op0=AT.add, op1=AT.mult
            )

            # hbT[h, j] = (v @ Wb)^T  for all local window columns
            sb_hbT = singles.tile([128, HC, W], f32)
            for hc in range(HC):
                ps = psh.tile([128, W], f32, tag="pre")
                for dc in range(DC):
                    nc.tensor.matmul(
                        ps,
                        sb_wb[:, dc, 128 * hc : 128 * (hc + 1)],
                        sb_vt16[:, dc, :],
                        start=(dc == 0),
                        stop=(dc == DC - 1),
                    )
                nc.vector.tensor_copy(out=sb_hbT[:, hc, :], in_=ps)

            # haTb[h, li] = (a_li @ Wa)^T + bp1
            sb_haTb = singles.tile([128, HC, R], f32)
            for hc in range(HC):
                ps = psh.tile([128, R], f32, tag="preA")
                for dc in range(DC):
                    nc.tensor.matmul(
                        ps,
                        sb_wa[:, dc, 128 * hc : 128 * (hc + 1)],
                        sb_vt16[:, dc, :R],
                        start=(dc == 0),
                        stop=(dc == DC - 1),
                    )
                nc.vector.tensor_scalar_add(
                    sb_haTb[:, hc, :], ps, sb_bp1[:, hc : hc + 1]
                )

            # main loop: pairs of rows, windows streamed as N=500 matmuls
            for p in range(R // 2):
                li0, li1 = 2 * p, 2 * p + 1
                rhs16 = rpool.tile([128, DC, 2 * K], f16)
                for dc in range(DC):
                    nc.vector.tensor_scalar_mul(
                        rhs16[:, dc, 0:K],
                        sb_vt[:, dc, li0 + 1 : li0 + 1 + K],
                        sb_vt[:, dc, li0 : li0 + 1],
                    )
                    nc.vector.tensor_scalar_mul(
                        rhs16[:, dc, K : 2 * K],
                        sb_vt[:, dc, li1 + 1 : li1 + 1 + K],
                        sb_vt[:, dc, li1 : li1 + 1],
                    )
                h16 = hpool.tile([128, HC, 2 * K], f16)
                for hc in range(HC):
                    ps = psh.tile([128, 2 * K], f32, tag="main")
                    for dc in range(DC):
                        nc.tensor.matmul(
                            ps,
                            sb_wab[:, dc, 128 * hc : 128 * (hc + 1)],
                            rhs16[:, dc, :],
                            start=(dc == 0),
                            stop=(dc == DC - 1),
                        )
                    hpre = tpool.tile([128, 2 * K], f32)
                    nc.vector.scalar_tensor_tensor(
                        hpre[:, 0:K],
                        ps[:, 0:K],
                        sb_haTb[:, hc, li0 : li0 + 1],
                        sb_hbT[:, hc, li0 + 1 : li0 + 1 + K],
                        op0=AT.add,
                        op1=AT.add,
                    )
                    nc.vector.scalar_tensor_tensor(
                        hpre[:, K : 2 * K],
                        ps[:, K : 2 * K],
                        sb_haTb[:, hc, li1 : li1 + 1],
                        sb_hbT[:, hc, li1 + 1 : li1 + 1 + K],
                        op0=AT.add,
                        op1=AT.add,
                    )
                    nc.scalar.activation(
                        out=h16[:, hc, :],
                        in_=hpre,
                        func=mybir.ActivationFunctionType.Relu,
                    )
                psr = psr_pool.tile([1, 2 * K], f32)
                for hc in range(HC):
                    nc.tensor.matmul(
                        psr,
                        sb_wp2[:, hc : hc + 1],
                        h16[:, hc, :],
                        start=(hc == 0),
                        stop=(hc == HC - 1),
                    )
                for li, off in ((li0, 0), (li1, K)):
                    tt = spool.tile([1, K + 1], f32, tag="tt")
                    nc.vector.tensor_add(
                        tt[:, 0:K], psr[:, off : off + K], sb_addt[0:1, li, :]
                    )
                    nc.vector.tensor_copy(
                        out=tt[:, K : K + 1], in_=sb_negd[0:1, li : li + 1]
                    )
                    mx = spool.tile([1, 1], f32, tag="mx")
                    nc.vector.reduce_max(
                        mx, tt, mybir.AxisListType.X, negate=True
                    )
                    ex = spool.tile([1, K + 1], f32, tag="ex")
                    sm = spool.tile([1, 1], f32, tag="sm")
                    nc.scalar.activation(
                        out=ex,
                        in_=tt,
                        func=mybir.ActivationFunctionType.Exp,
                        bias=mx[0:1, 0:1],
                        scale=1.0,
                        accum_out=sm,
                    )
                    rs = spool.tile([1, 1], f32, tag="rs")
                    nc.vector.reciprocal(out=rs, in_=sm)
                    oo = opool.tile([1, K + 1], f32)
                    nc.vector.tensor_scalar_mul(oo, ex, rs[0:1, 0:1])
                    nc.sync.dma_start(out=out.ap()[li : li + 1, :], in_=oo)
    nc.compile()
    return nc


def _chunked(a2d):
    """[D, F] -> [128, D//128, F] host layout (partition-major for SBUF)."""
    dd, ff = a2d.shape
    return np.ascontiguousarray(a2d.reshape(dd // 128, 128, ff).transpose(1, 0, 2))


def _hcols(vec):
    """[H] -> [128, H//128]: column hc holds vec[128*hc : 128*(hc+1)]."""
    return np.ascontiguousarray(vec.reshape(-1, 128).T)


def _run(nc, in_maps):
    from concourse import bass_utils

    return bass_utils.run_bass_kernel_spmd(nc, in_maps, core_ids=list(range(NCORES)))


def kernel(
    vectors,
    Wm1,
    bm1,
    wm2,
    bm2,
    Wp1,
    bp1,
    wp2,
    bp2,
    span_starts,
    span_ends,
    t,
):
    vectors = np.asarray(vectors, dtype=np.float32)
    Wm1 = np.asarray(Wm1, dtype=np.float32)
    bm1 = np.asarray(bm1, dtype=np.float32)
    wm2 = np.asarray(wm2, dtype=np.float32)
    bm2 = np.asarray(bm2, dtype=np.float32)
    Wp1 = np.asarray(Wp1, dtype=np.float32)
    bp1 = np.asarray(bp1, dtype=np.float32)
    wp2 = np.asarray(wp2, dtype=np.float32)
    bp2 = np.asarray(bp2, dtype=np.float32)
    span_starts = np.asarray(span_starts)
    span_ends = np.asarray(span_ends)
    m = int(0.4 * int(np.asarray(t)))
    assert m == M and vectors.shape == (N_SPANS, D)

    # ---- NEFF 1: mention scores, spans sharded 8 x 250 ----
    if "sc" not in _CACHE:
        _CACHE["sc"] = build_scores_nc()
    nc1 = _CACHE["sc"]
    wm1_h = _chunked(Wm1.astype(np.float32))
    bm1_h = _hcols(bm1)
    wm2_h = _hcols(wm2)
    bm2_h = bm2.reshape(1, 1)
    vT = np.ascontiguousarray(vectors.T)  # [768, 2000]
    in_maps1 = []
    for c in range(NCORES):
        in_maps1.append(
            {
                "vt": _chunked(vT[:, c * NS : (c + 1) * NS]),
                "wm1": wm1_h,
                "bm1h": bm1_h,
                "wm2h": wm2_h,
                "bm2s": bm2_h,
            }
        )
    t0 = time.time()
    res1 = _run(nc1, in_maps1)
    LAST_INFO["scores_wall"] = time.time() - t0
    scores = np.concatenate([res1.results[c]["out"][0] for c in range(NCORES)])

    # ---- host: top-k selection + (start, end) descending re-sort ----
    top = np.argsort(-scores, kind="stable")[:m]
    order = np.lexsort((span_ends[top], span_starts[top]))[::-1]
    sel = top[order]
    LAST_INFO["scores"] = scores
    LAST_INFO["sel"] = sel
    v_s = vectors[sel]              # [400, 768]
    s_s = scores[sel].astype(np.float32)

    # ---- NEFF 2: pairwise + softmax, rows sharded 8 x 50 ----
    if "pw" not in _CACHE:
        _CACHE["pw"] = build_pair_nc()
    nc2 = _CACHE["pw"]
    wa_h = _chunked(Wp1[:D].astype(np.float16))
    wb_h = _chunked(Wp1[D : 2 * D].astype(np.float16))
    wab_h = _chunked(Wp1[2 * D :].astype(np.float16))
    bp1_h = _hcols(bp1)
    wp2_h = _hcols(wp2).astype(np.float16)
    bp2_h = bp2.reshape(1, 1)

    vsT_pad = np.zeros((D, NCORES * R + W), np.float32)
    vsT_pad[:, :m] = v_s.T
    s_pad = np.zeros(NCORES * R + W, np.float32)
    s_pad[:m] = s_s

    in_maps2 = []
    for c in range(NCORES):
        base = c * R
        addt = np.full((R, K), NEG, np.float32)
        for li in range(R):
            i = base + li
            nvalid = max(0, min(K, (m - 1) - i))
            if nvalid:
                addt[li, :nvalid] = s_pad[i + 1 : i + 1 + nvalid]
        in_maps2.append(
            {
                "vt": _chunked(vsT_pad[:, base : base + W]),
                "wab": wab_h,
                "wa": wa_h,
                "wb": wb_h,
                "bp1h": bp1_h,
                "wp2h": wp2_h,
                "bp2s": bp2_h,
                "srow": s_pad[base : base + R].reshape(1, R),
                "addt": addt.reshape(1, R, K),
            }
        )
    t0 = time.time()
    res2 = _run(nc2, in_maps2)
    LAST_INFO["pair_wall"] = time.time() - t0

    out = np.empty((m - 1, K + 1), np.float32)
    for c in range(NCORES):
        lo = c * R
        hi = min(lo + R, m - 1)
        out[lo:hi] = res2.results[c]["out"][: hi - lo]
    return out


# revision 29
# speedup vs baseline: 1.0730x; 1.0730x over previous
"""Trainium2 Bass kernel for nn_CoreferenceModel (sparse_attention).

Strategy (8 NeuronCores, SPMD, no collectives):
  NEFF 1 (mention scorer): shard the N=2000 spans 8x250. Each core computes
    scores = relu(v @ Wm1 + bm1) @ wm2 + bm2 for its 250 spans (fp32 matmuls,
    D on the contraction/partition axis). Host gathers [2000] scores.
  Host: top-k (m=400) by score + lexsort by (start, end) descending —
    tiny integer work that is inherently sequential.
  NEFF 2 (pairwise scorer + softmax): shard the 399 output rows 8x50. The
    antecedent window of row i is the contiguous span block i+1..i+250 in
    sorted order, so each core gets a 300-column slice of v^T and computes
      h   = relu((a@Wa) + (v@Wb)[win] + (a*b)@Wab + bp1)   (fp16 matmuls)
      ant = h @ wp2 (+ shift-invariant constants) ; softmax over [250 + dummy]
    with (v@Wb), (a@Wa)+bp1 precomputed per-core, windows as pure slices.
    Rows are processed in pairs so the big matmuls stream N=500 columns.

Softmax shift trick: softmax(x + c) == softmax(x), so the per-row constant
(s_i + bp2) is not added to the 250 antecedent logits; instead the dummy
column becomes -(s_i + bp2). s-window values arrive pre-gathered (+ -1e30
mask) as a host-staged tensor; the adds happen on-device.
"""

import time

import numpy as np

D = 768
H = 1024
K = 250
N_SPANS = 2000
NCORES = 8
M = 400            # int(0.4 * 1000) retained spans
R = 50             # output rows per core (last core's row 399 is discarded)
W = R + K          # v^T columns per core in NEFF 2
NS = N_SPANS // NCORES  # spans per core in NEFF 1
DC = D // 128      # contraction chunks
HC = H // 128      # hidden chunks
NEG = -1.0e30

LAST_INFO: dict = {}
_CACHE: dict = {}


def _mybir():
    from concourse import mybir

    return mybir


def build_scores_nc():
    import concourse.bacc as bacc
    import concourse.tile as tile

    mybir = _mybir()
    f32 = mybir.dt.float32

    nc = bacc.Bacc("TRN2", target_bir_lowering=False)
    vt = nc.dram_tensor("vt", [128, DC, NS], f32, kind="ExternalInput")
    wm1 = nc.dram_tensor("wm1", [128, DC, H], f32, kind="ExternalInput")
    bm1h = nc.dram_tensor("bm1h", [128, HC], f32, kind="ExternalInput")
    wm2h = nc.dram_tensor("wm2h", [128, HC], f32, kind="ExternalInput")
    bm2s = nc.dram_tensor("bm2s", [1, 1], f32, kind="ExternalInput")
    out = nc.dram_tensor("out", [1, NS], f32, kind="ExternalOutput")

    with tile.TileContext(nc) as tc:
        with (
            tc.tile_pool(name="singles", bufs=1) as singles,
            tc.tile_pool(name="work", bufs=2) as work,
            tc.tile_pool(name="psh", bufs=2, space="PSUM") as psh,
            tc.tile_pool(name="pss", bufs=1, space="PSUM") as pss,
        ):
            sb_v = singles.tile([128, DC, NS], f32)
            nc.sync.dma_start(out=sb_v, in_=vt.ap())
            sb_bm1 = singles.tile([128, HC], f32)
            nc.sync.dma_start(out=sb_bm1, in_=bm1h.ap())
            sb_wm2 = singles.tile([128, HC], f32)
            nc.sync.dma_start(out=sb_wm2, in_=wm2h.ap())
            sb_bm2 = singles.tile([1, 1], f32)
            nc.sync.dma_start(out=sb_bm2, in_=bm2s.ap())
            # per-hc weight DMA: the hc=0 matmuls only wait for 1/8 of Wm1
            sb_w = singles.tile([128, DC, H], f32)
            for hc in range(HC):
                nc.sync.dma_start(
                    out=sb_w[:, :, 128 * hc : 128 * (hc + 1)],
                    in_=wm1.ap()[:, :, 128 * hc : 128 * (hc + 1)],
                )

            # wm2-dot matmul for chunk hc issued right after its relu, so the
            # ps2 accumulation hides under later chunks' hidden-layer matmuls
            h_sb = singles.tile([128, HC, NS], f32)
            ps2 = pss.tile([1, NS], f32)
            for hc in range(HC):
                ps = psh.tile([128, NS], f32)
                for dc in range(DC):
                    nc.tensor.matmul(
                        ps,
                        sb_w[:, dc, 128 * hc : 128 * (hc + 1)],
                        sb_v[:, dc, :],
                        start=(dc == 0),
                        stop=(dc == DC - 1),
                    )
                nc.scalar.activation(
                    out=h_sb[:, hc, :],
                    in_=ps,
                    func=mybir.ActivationFunctionType.Relu,
                    bias=sb_bm1[:, hc : hc + 1],
                    scale=1.0,
                )
                nc.tensor.matmul(
                    ps2,
                    sb_wm2[:, hc : hc + 1],
                    h_sb[:, hc, :],
                    start=(hc == 0),
                    stop=(hc == HC - 1),
                )
            o_sb = work.tile([1, NS], f32)
            nc.vector.tensor_scalar_add(o_sb, ps2, sb_bm2[0:1, 0:1])
            nc.sync.dma_start(out=out.ap(), in_=o_sb)
    nc.compile()
    return nc


def build_pair_nc():
    import concourse.bacc as bacc
    import concourse.tile as tile

    mybir = _mybir()
    f32 = mybir.dt.float32
    f16 = mybir.dt.float16
    AT = mybir.AluOpType

    nc = bacc.Bacc("TRN2", target_bir_lowering=False)
    vt = nc.dram_tensor("vt", [128, DC, W], f32, kind="ExternalInput")
    wab = nc.dram_tensor("wab", [128, DC, H], f16, kind="ExternalInput")
    wa = nc.dram_tensor("wa", [128, DC, H], f16, kind="ExternalInput")
    wb = nc.dram_tensor("wb", [128, DC, H], f16, kind="ExternalInput")
    bp1h = nc.dram_tensor("bp1h", [128, HC], f32, kind="ExternalInput")
    wp2h = nc.dram_tensor("wp2h", [128, HC], f16, kind="ExternalInput")
    bp2s = nc.dram_tensor("bp2s", [1, 1], f32, kind="ExternalInput")
    srow = nc.dram_tensor("srow", [1, R], f32, kind="ExternalInput")
    addt = nc.dram_tensor("addt", [1, R, K], f32, kind="ExternalInput")
    out = nc.dram_tensor("out", [R, K + 1], f32, kind="ExternalOutput")

    with tile.TileContext(nc) as tc:
        with (
            tc.tile_pool(name="singles", bufs=1) as singles,
            tc.tile_pool(name="rpool", bufs=3) as rpool,
            tc.tile_pool(name="hpool", bufs=2) as hpool,
            tc.tile_pool(name="tpool", bufs=3) as tpool,
            tc.tile_pool(name="spool", bufs=3) as spool,
            tc.tile_pool(name="opool", bufs=4) as opool,
            tc.tile_pool(name="psh", bufs=5, space="PSUM") as psh,
            tc.tile_pool(name="psr", bufs=2, space="PSUM") as psr_pool,
        ):
            # DMA order = consumption order: vt feeds everything, wb/wa feed
            # the precompute, wab is only needed once the main loop starts.
            sb_vt = singles.tile([128, DC, W], f32)
            nc.sync.dma_start(out=sb_vt, in_=vt.ap())
            sb_addt = singles.tile([1, R, K], f32)
            nc.sync.dma_start(out=sb_addt, in_=addt.ap())
            sb_wb = singles.tile([128, DC, H], f16)
            nc.sync.dma_start(out=sb_wb, in_=wb.ap())
            sb_wa = singles.tile([128, DC, H], f16)
            nc.sync.dma_start(out=sb_wa, in_=wa.ap())
            sb_bp1 = singles.tile([128, HC], f32)
            nc.sync.dma_start(out=sb_bp1, in_=bp1h.ap())
            sb_wp2 = singles.tile([128, HC], f16)
            nc.sync.dma_start(out=sb_wp2, in_=wp2h.ap())
            sb_bp2 = singles.tile([1, 1], f32)
            nc.sync.dma_start(out=sb_bp2, in_=bp2s.ap())
            sb_srow = singles.tile([1, R], f32)
            nc.sync.dma_start(out=sb_srow, in_=srow.ap())
            sb_wab = singles.tile([128, DC, H], f16)
            nc.sync.dma_start(out=sb_wab, in_=wab.ap())

            # fp16 copy of v^T for the precompute matmuls
            sb_vt16 = singles.tile([128, DC, W], f16)
            for dc in range(DC):
                nc.vector.tensor_copy(out=sb_vt16[:, dc, :], in_=sb_vt[:, dc, :])

            # negd[li] = -(s_i + bp2): value of the dummy column after the
            # shift-invariant removal of (s_i + bp2) from every logit.
            sb_negd = singles.tile([1, R], f32)
            nc.vector.tensor_scalar(
                sb_negd, sb_srow, sb_bp2[0:1, 0:1], -1.0, op0=AT.add, op1=AT.mult
            )

            # hbT[h, j] = (v @ Wb)^T  for all local window columns
            sb_hbT = singles.tile([128, HC, W], f32)
            for hc in range(HC):
                ps = psh.tile([128, 2 * K], f32, tag="ps", name="ps_pre")[:, :W]
                for dc in range(DC):
                    nc.tensor.matmul(
                        ps,
                        sb_wb[:, dc, 128 * hc : 128 * (hc + 1)],
                        sb_vt16[:, dc, :],
                        start=(dc == 0),
                        stop=(dc == DC - 1),
                    )
                nc.vector.tensor_copy(out=sb_hbT[:, hc, :], in_=ps)

            # haTb[h, li] = (a_li @ Wa)^T + bp1
            sb_haTb = singles.tile([128, HC, R], f32)
            for hc in range(HC):
                ps = psh.tile([128, 2 * K], f32, tag="ps", name="ps_preA")[:, :R]
                for dc in range(DC):
                    nc.tensor.matmul(
                        ps,
                        sb_wa[:, dc, 128 * hc : 128 * (hc + 1)],
                        sb_vt16[:, dc, :R],
                        start=(dc == 0),
                        stop=(dc == DC - 1),
                    )
                nc.vector.tensor_scalar_add(
                    sb_haTb[:, hc, :], ps, sb_bp1[:, hc : hc + 1]
                )

            # main loop: pairs of rows, windows streamed as N=500 matmuls
            for p in range(R // 2):
                li0, li1 = 2 * p, 2 * p + 1
                # window scaling on GPSIMD (otherwise idle; frees DVE)
                rhs16 = rpool.tile([128, DC, 2 * K], f16)
                for dc in range(DC):
                    nc.gpsimd.tensor_scalar_mul(
                        rhs16[:, dc, 0:K],
                        sb_vt[:, dc, li0 + 1 : li0 + 1 + K],
                        sb_vt[:, dc, li0 : li0 + 1],
                    )
                    nc.gpsimd.tensor_scalar_mul(
                        rhs16[:, dc, K : 2 * K],
                        sb_vt[:, dc, li1 + 1 : li1 + 1 + K],
                        sb_vt[:, dc, li1 : li1 + 1],
                    )
                h16 = hpool.tile([128, HC, 2 * K], f16)
                for hc in range(HC):
                    ps = psh.tile([128, 2 * K], f32, tag="ps", name="ps_main")
                    for dc in range(DC):
                        nc.tensor.matmul(
                            ps,
                            sb_wab[:, dc, 128 * hc : 128 * (hc + 1)],
                            rhs16[:, dc, :],
                            start=(dc == 0),
                            stop=(dc == DC - 1),
                        )
                    hpre = tpool.tile([128, 2 * K], f32)
                    nc.vector.scalar_tensor_tensor(
                        hpre[:, 0:K],
                        ps[:, 0:K],
                        sb_haTb[:, hc, li0 : li0 + 1],
                        sb_hbT[:, hc, li0 + 1 : li0 + 1 + K],
                        op0=AT.add,
                        op1=AT.add,
                    )
                    nc.vector.scalar_tensor_tensor(
                        hpre[:, K : 2 * K],
                        ps[:, K : 2 * K],
                        sb_haTb[:, hc, li1 : li1 + 1],
                        sb_hbT[:, hc, li1 + 1 : li1 + 1 + K],
                        op0=AT.add,
                        op1=AT.add,
                    )
                    nc.scalar.activation(
                        out=h16[:, hc, :],
                        in_=hpre,
                        func=mybir.ActivationFunctionType.Relu,
                    )
                # wp2 dots kept AFTER the full hc loop: a wp2 matmul depends
                # on ACT's relu, and the in-order PE queue would stall the
                # next chunk's (independent) einsum matmuls behind it
                psr = psr_pool.tile([1, 2 * K], f32)
                for hc in range(HC):
                    nc.tensor.matmul(
                        psr,
                        sb_wp2[:, hc : hc + 1],
                        h16[:, hc, :],
                        start=(hc == 0),
                        stop=(hc == HC - 1),
                    )
                # logits for both rows in one op: clamp (overflow guard, never
                # binding at these scales) fused with the s-window add; the
                # per-row constant (s_i + bp2) is dropped (softmax shift
                # invariance) and the dummy column becomes -(s_i + bp2).
                tt = spool.tile([1, 2, K + 1], f32, tag="tt")
                nc.vector.scalar_tensor_tensor(
                    tt[:, :, 0:K],
                    psr.rearrange("p (a b) -> p a b", a=2),
                    60.0,
                    sb_addt[0:1, li0 : li0 + 2, :],
                    op0=AT.min,
                    op1=AT.add,
                )
                nc.gpsimd.tensor_copy(
                    out=tt[:, :, K : K + 1],
                    in_=sb_negd[0:1, li0 : li0 + 2].rearrange(
                        "p (a b) -> p a b", b=1
                    ),
                )
                sm = spool.tile([1, 2], f32, tag="sm")
                ex = spool.tile([1, 2, K + 1], f32, tag="ex")
                for j in (0, 1):
                    nc.scalar.activation(
                        out=ex[:, j, :],
                        in_=tt[:, j, :],
                        func=mybir.ActivationFunctionType.Exp,
                        accum_out=sm[0:1, j : j + 1],
                    )
                rs = spool.tile([1, 2], f32, tag="rs")
                nc.vector.reciprocal(out=rs, in_=sm)
                for j, li in ((0, li0), (1, li1)):
                    oo = opool.tile([1, K + 1], f32)
                    nc.gpsimd.tensor_scalar_mul(oo, ex[:, j, :], rs[0:1, j : j + 1])
                    nc.sync.dma_start(out=out.ap()[li : li + 1, :], in_=oo)
    nc.compile()
    return nc


def _chunked(a2d):
    """[D, F] -> [128, D//128, F] host layout (partition-major for SBUF)."""
    dd, ff = a2d.shape
    return np.ascontiguousarray(a2d.reshape(dd // 128, 128, ff).transpose(1, 0, 2))


def _hcols(vec):
    """[H] -> [128, H//128]: column hc holds vec[128*hc : 128*(hc+1)]."""
    return np.ascontiguousarray(vec.reshape(-1, 128).T)


def _run(nc, in_maps):
    from concourse import bass_utils

    return bass_utils.run_bass_kernel_spmd(nc, in_maps, core_ids=list(range(NCORES)))


def kernel(
    vectors,
    Wm1,
    bm1,
    wm2,
    bm2,
    Wp1,
    bp1,
    wp2,
    bp2,
    span_starts,
    span_ends,
    t,
):
    vectors = np.asarray(vectors, dtype=np.float32)
    Wm1 = np.asarray(Wm1, dtype=np.float32)
    bm1 = np.asarray(bm1, dtype=np.float32)
    wm2 = np.asarray(wm2, dtype=np.float32)
    bm2 = np.asarray(bm2, dtype=np.float32)
    Wp1 = np.asarray(Wp1, dtype=np.float32)
    bp1 = np.asarray(bp1, dtype=np.float32)
    wp2 = np.asarray(wp2, dtype=np.float32)
    bp2 = np.asarray(bp2, dtype=np.float32)
    span_starts = np.asarray(span_starts)
    span_ends = np.asarray(span_ends)
    m = int(0.4 * int(np.asarray(t)))
    assert m == M and vectors.shape == (N_SPANS, D)

    # ---- NEFF 1: mention scores, spans sharded 8 x 250 ----
    if "sc" not in _CACHE:
        _CACHE["sc"] = build_scores_nc()
    nc1 = _CACHE["sc"]
    wm1_h = _chunked(Wm1.astype(np.float32))
    bm1_h = _hcols(bm1)
    wm2_h = _hcols(wm2)
    bm2_h = bm2.reshape(1, 1)
    vT = np.ascontiguousarray(vectors.T)  # [768, 2000]
    in_maps1 = []
    for c in range(NCORES):
        in_maps1.append(
            {
                "vt": _chunked(vT[:, c * NS : (c + 1) * NS]),
                "wm1": wm1_h,
                "bm1h": bm1_h,
                "wm2h": wm2_h,
                "bm2s": bm2_h,
            }
        )
    t0 = time.time()
    res1 = _run(nc1, in_maps1)
    LAST_INFO["scores_wall"] = time.time() - t0
    scores = np.concatenate([res1.results[c]["out"][0] for c in range(NCORES)])

    # ---- host: top-k selection + (start, end) descending re-sort ----
    top = np.argsort(-scores, kind="stable")[:m]
    order = np.lexsort((span_ends[top], span_starts[top]))[::-1]
    sel = top[order]
    LAST_INFO["scores"] = scores
    LAST_INFO["sel"] = sel
    v_s = vectors[sel]              # [400, 768]
    s_s = scores[sel].astype(np.float32)

    # ---- NEFF 2: pairwise + softmax, rows sharded 8 x 50 ----
    if "pw" not in _CACHE:
        _CACHE["pw"] = build_pair_nc()
    nc2 = _CACHE["pw"]
    wa_h = _chunked(Wp1[:D].astype(np.float16))
    wb_h = _chunked(Wp1[D : 2 * D].astype(np.float16))
    wab_h = _chunked(Wp1[2 * D :].astype(np.float16))
    bp1_h = _hcols(bp1)
    wp2_h = _hcols(wp2).astype(np.float16)
    bp2_h = bp2.reshape(1, 1)

    vsT_pad = np.zeros((D, NCORES * R + W), np.float32)
    vsT_pad[:, :m] = v_s.T
    s_pad = np.zeros(NCORES * R + W, np.float32)
    s_pad[:m] = s_s

    in_maps2 = []
    for c in range(NCORES):
        base = c * R
        addt = np.full((R, K), NEG, np.float32)
        for li in range(R):
            i = base + li
            nvalid = max(0, min(K, (m - 1) - i))
            if nvalid:
                addt[li, :nvalid] = s_pad[i + 1 : i + 1 + nvalid]
        in_maps2.append(
            {
                "vt": _chunked(vsT_pad[:, base : base + W]),
                "wab": wab_h,
                "wa": wa_h,
                "wb": wb_h,
                "bp1h": bp1_h,
                "wp2h": wp2_h,
                "bp2s": bp2_h,
                "srow": s_pad[base : base + R].reshape(1, R),
                "addt": addt.reshape(1, R, K),
            }
        )
    t0 = time.time()
    res2 = _run(nc2, in_maps2)
    LAST_INFO["pair_wall"] = time.time() - t0

    out = np.empty((m - 1, K + 1), np.float32)
    for c in range(NCORES):
        lo = c * R
        hi = min(lo + R, m - 1)
        out[lo:hi] = res2.results[c]["out"][: hi - lo]
    return out
